# revision 31
# baseline (speedup 1.0000x reference)
"""Localized embedding layer (Gaussian stencil) on 8 trn2 cores — bf16 pipeline.

Math: out[i,j,:] = sum_{|di|,|dj|<=2} w(di)w(dj) H[i+di,j+dj,:] / (r(i)*r(j))
with w(d) = exp(-c*d^2), c = TILE^2/(2 sigma^2). w(2) ~ 4.4e-5 is far below
the error budget, so the numerator collapses to a 3x3 stencil (the r(i)r(j)
normalizer keeps the exact 5-tap sums of the reference).

Sharding: 32 grid rows per core + 1-row halo each side (zero padded at the
global edges). All device I/O is bf16 (~0.3% rms error, budget 2e-2): this
problem is memory-bound and bf16 halves HBM traffic.

Layout: one grid row = one SBUF tile [128 partitions, 2 cells x 512 feat];
partition p holds cells 2p, 2p+1 (host pre-transposes shards to
partition-major [128, rows, 1024] so DMA lines are contiguous).

The j-conv runs on the PE as 128x128 matmuls whose matrices carry the exact
1/(w_full*r_col(j)) normalizer (so grid-edge columns need no fix pass); the
per-row normalizer w_full/r_row(i) (!=1 only for global rows 0,1,254,255) is
baked into dedicated per-core matrix sets for those rows, so the PSUM->SBUF
copy on ACT is scale-free and can process two rows per instruction.

The i-conv is balanced between DVE and PE per 4-row "quad":
  D-quads: DVE computes tt = (u1*x)[i-1] + (u1*x)[i+1] + x[i] as two packed
    2x-mode TTs over a prescaled z = u1*x chunk (avoids the 1x-mode STT);
    4 matmuls/row.
  P-quads: DVE computes only q = x[i-1]+x[i+1]; the PE folds u1 via extra
    matrix sets and consumes q and x[i] directly; 8 matmuls/row.
5 D / 3 P quads puts DVE and PE both at ~38us, under the ~45us DMA stream.
"""

import sys
import numpy as np

if "/opt/trn_rl_repo" not in sys.path:
    sys.path.insert(0, "/opt/trn_rl_repo")

G = 256          # grid side
D = 512          # feature dim
NC = 8           # cores
RPC = G // NC    # output rows per core = 32
NR = RPC + 2     # input rows per core incl 1-row halo = 34
G2 = 2 * D       # free size of one grid row tile (2 cells x 512)
TILE = 448.0
SIGMA = 200.0
P5 = 2           # reference stencil half-width (for the normalizer r)
# matrix sets: std, u1*std, row0, row1, row30, row31, u1*row0, u1*row1,
# u1*row30, u1*row31
NSET = 10

# input DMA row ranges in the 34-row shard (first one tiny so the very first
# compute — a 2-row P piece needing only rows 0..3 — starts ~2us earlier);
# subtile dependency tracking lets every consumer wait on exactly the loads
# covering its rows, so ranges don't overlap.
LOADS = [(0, 4), (4, 12), (12, 20), (20, 28), (28, 34)]
# P-quads first: their raw TT needs no z-prescale, so the PE pipeline kicks
# off as early as possible. (Making the LAST quad P-scheme was tried and is
# ~8us WORSE: the +32 matmuls grow the PE stream by more than the drain
# saves.)
SCHEME = ["P", "P", "D", "D", "D", "D", "D", "D"]
# z = u1*x ranges (cover side-tap rows of D-quads only), split at LOAD
# boundaries so no TS op waits on a later DMA than its consumers need; the
# z tile holds rows 8..33 at index row-8.
ZRANGES = [(8, 12), (12, 20), (20, 28), (28, 34)]
ZROWS = 26


def _zi(r):
    return r - 8

_cache = {}


def _weights5():
    c = TILE * TILE / (2.0 * SIGMA * SIGMA)
    return np.exp(-c * np.arange(-P5, P5 + 1) ** 2)   # [w2,w1,1,w1,w2] f64


def _r_vec():
    """r(i) = sum of valid 5-tap weights at row/col i (reference normalizer)."""
    w = _weights5()
    r = np.zeros(G)
    for d in range(-P5, P5 + 1):
        lo, hi = max(0, -d), min(G, G - d)
        r[lo:hi] += w[d + P5]
    return r


def _host_consts():
    import ml_dtypes

    w = _weights5()
    u1 = float(w[1])
    r = _r_vec()
    wf = float(w.sum())
    # Base j-conv matrices, m[q, k, p]: ps_c[p] = sum_q m[q,k(c),p] * rhs[q]
    base = np.zeros((128, 4, 128))
    for p in range(128):
        s0 = 1.0 / (wf * r[2 * p])
        s1 = 1.0 / (wf * r[2 * p + 1])
        base[p, 0, p] = s0                      # D0: c0 center
        base[p, 1, p] = u1 * s0                 # B: c0 <- c1 (cell 2p+1)
        if p > 0:
            base[p - 1, 1, p] = u1 * s0         # B: c0 <- c1 (cell 2p-1)
        base[p, 2, p] = u1 * s1                 # C: c1 <- c0 (cell 2p)
        if p < 127:
            base[p + 1, 2, p] = u1 * s1         # C: c1 <- c0 (cell 2p+2)
        base[p, 3, p] = s1                      # D1: c1 center
    # Per-core matrix sets: [std, u1*std, s_row0*std, s_row1*std,
    #                        s_row30*std, s_row31*std, u1*s_row0*std,
    #                        u1*s_row1*std]
    wmats = []
    for c in range(NC):
        sets = [base, u1 * base]
        for i in (0, 1, RPC - 2, RPC - 1):
            s = wf / r[RPC * c + i]             # ==1 away from global edges
            sets.append(s * base)
        for i in (0, 1, RPC - 2, RPC - 1):
            s = wf / r[RPC * c + i]
            sets.append(u1 * s * base)
        wm = np.concatenate(sets, axis=1)        # [128, 40, 128]
        wmats.append(wm.astype(ml_dtypes.bfloat16))
    return u1, wmats


def _kset(i):
    """Matrix-set index for output row i (0=std; edge rows pre-scaled)."""
    return {0: 2, 1: 3, RPC - 2: 4, RPC - 1: 5}.get(i, 0)


def _build_nc(u1):
    import concourse.bass as bass  # noqa: F401
    import concourse.mybir as mybir
    import concourse.tile as tile
    from concourse import bacc

    f32 = mybir.dt.float32
    bf16 = mybir.dt.bfloat16
    add = mybir.AluOpType.add

    nc = bacc.Bacc(None, target_bir_lowering=False, debug=False)
    x_dram = nc.declare_dram_parameter("x", [128, NR, G2], bf16, isOutput=False)
    wm_dram = nc.declare_dram_parameter(
        "wmat", [128, 4 * NSET, 128], bf16, isOutput=False
    )
    y_dram = nc.declare_dram_parameter("y", [128, RPC, G2], bf16, isOutput=True)

    with tile.TileContext(nc) as tc:
        with (
            tc.tile_pool(name="const", bufs=1) as cpool,
            tc.tile_pool(name="x", bufs=1) as xpool,
            tc.tile_pool(name="z", bufs=1) as zpool,
            tc.tile_pool(name="qp", bufs=4) as qpool,
            tc.tile_pool(name="ttp", bufs=3) as ttpool,
            tc.tile_pool(name="out", bufs=2) as opool,
            tc.tile_pool(name="psum", bufs=2, space="PSUM") as ppool,
        ):
            # consts ride the scalar ring so they don't delay the first load
            wt = cpool.tile([128, 4 * NSET, 128], bf16)
            nc.scalar.dma_start(wt[:], wm_dram[:])

            xt = xpool.tile([128, NR, G2], bf16, name="xt")
            for s, e in LOADS:
                nc.sync.dma_start(xt[:, s:e, :], x_dram[:, s:e, :])
            zt = zpool.tile([128, ZROWS, G2], bf16, name="zt")
            for s, e in ZRANGES:
                nc.vector.tensor_scalar_mul(
                    zt[:, _zi(s):_zi(s) + (e - s), :], xt[:, s:e, :], u1
                )

            def d_pair(ob, ttq, tb, q, mm, dve_copy=False):
                """ttq rows [tb, tb+1] -> output rows 4q+mm, 4q+mm+1."""
                ps = ppool.tile([128, 2 * G2], f32, tag="ps", name="ps")
                for rr in (0, 1):
                    k = 4 * _kset(4 * q + mm + rr)
                    o = rr * G2
                    rhs0 = ttq[:, tb + rr, 0:D]
                    rhs1 = ttq[:, tb + rr, D:G2]
                    nc.tensor.matmul(
                        ps[:, o:o + D], wt[:, k + 0, :], rhs0,
                        start=True, stop=False,
                    )
                    nc.tensor.matmul(
                        ps[:, o:o + D], wt[:, k + 1, :], rhs1,
                        start=False, stop=True,
                    )
                    nc.tensor.matmul(
                        ps[:, o + D:o + G2], wt[:, k + 2, :], rhs0,
                        start=True, stop=False,
                    )
                    nc.tensor.matmul(
                        ps[:, o + D:o + G2], wt[:, k + 3, :], rhs1,
                        start=False, stop=True,
                    )
                if dve_copy:
                    # DVE is idle during the drain; halves the ACT tail
                    nc.vector.tensor_copy(ob[:, mm:mm + 2, :], ps[:])
                else:
                    nc.scalar.copy(ob[:, mm:mm + 2, :], ps[:])

            def d_ttq(b):
                """DVE i-conv for a D-quad starting at input row b."""
                zb = _zi(b)
                qt = qpool.tile([128, 4, G2], bf16, tag="qt", name="qt")
                nc.vector.tensor_tensor(
                    qt[:], zt[:, zb:zb + 4, :], zt[:, zb + 2:zb + 6, :], add
                )
                ttq = ttpool.tile([128, 4, G2], bf16, tag="tt", name="ttq")
                nc.vector.tensor_tensor(
                    ttq[:], qt[:], xt[:, b + 1:b + 5, :], add
                )
                return ttq

            def p_pair(ob, qt, tb, i0, om):
                """P-scheme: qt rows [tb, tb+1] -> output rows i0, i0+1
                (written to ob rows om, om+1)."""
                ps = ppool.tile([128, 2 * G2], f32, tag="ps", name="ps")
                for rr in (0, 1):
                    i = i0 + rr
                    kc = 4 * _kset(i)              # center sets
                    ku = {0: 24, 1: 28, RPC - 2: 32, RPC - 1: 36}.get(i, 4)
                    o = rr * G2
                    qa, qb = qt[:, tb + rr, 0:D], qt[:, tb + rr, D:G2]
                    x1 = xt[:, i + 1, :]
                    for (lo, hi), s in (((0, D), 0), ((D, G2), 2)):
                        nc.tensor.matmul(
                            ps[:, o + lo:o + hi], wt[:, ku + s, :], qa,
                            start=True, stop=False,
                        )
                        nc.tensor.matmul(
                            ps[:, o + lo:o + hi], wt[:, ku + 1 + s, :],
                            qb, start=False, stop=False,
                        )
                        nc.tensor.matmul(
                            ps[:, o + lo:o + hi], wt[:, kc + s, :],
                            x1[:, 0:D], start=False, stop=False,
                        )
                        nc.tensor.matmul(
                            ps[:, o + lo:o + hi], wt[:, kc + 1 + s, :],
                            x1[:, D:G2], start=False, stop=True,
                        )
                nc.scalar.copy(ob[:, om:om + 2, :], ps[:])

            for q in range(8):
                b = 4 * q                      # first input row of the quad
                if q == 0:
                    # first quad in 2-row pieces: the first piece needs only
                    # input rows 0..3 (the tiny first load), so PE starts
                    # ~2us earlier
                    ob = opool.tile([128, 4, G2], bf16, tag="ob", name="ob")
                    for mm in (0, 2):
                        qt = qpool.tile([128, 2, G2], bf16, tag="qt", name="qt")
                        nc.vector.tensor_tensor(
                            qt[:], xt[:, mm:mm + 2, :], xt[:, mm + 2:mm + 4, :],
                            add,
                        )
                        p_pair(ob, qt, 0, mm, mm)
                    nc.scalar.dma_start(y_dram[:, 0:4, :], ob[:])
                    continue
                if q == 7:
                    # final quad: per-pair stores (last on the idle sync ring)
                    # shorten the drain tail
                    ttq = d_ttq(b)
                    ob7 = opool.tile([128, 4, G2], bf16, tag="ob", name="ob")
                    d_pair(ob7, ttq, 0, 7, 0)
                    nc.scalar.dma_start(y_dram[:, 28:30, :], ob7[:, 0:2, :])
                    d_pair(ob7, ttq, 2, 7, 2)
                    nc.sync.dma_start(y_dram[:, 30:32, :], ob7[:, 2:4, :])
                    break
                ob = opool.tile([128, 4, G2], bf16, tag="ob", name="ob")
                if SCHEME[q] == "D":
                    ttq = d_ttq(b)
                    for mm in (0, 2):
                        d_pair(ob, ttq, mm, q, mm)
                    nc.scalar.dma_start(y_dram[:, b:b + 4, :], ob[:])
                else:
                    # P-quad: q_raw = x[i-1] + x[i+1]; PE folds u1 and center
                    qt = qpool.tile([128, 4, G2], bf16, tag="qt", name="qt")
                    nc.vector.tensor_tensor(
                        qt[:], xt[:, b:b + 4, :], xt[:, b + 2:b + 6, :], add
                    )
                    for mm in (0, 2):
                        p_pair(ob, qt, mm, 4 * q + mm, mm)
                    nc.scalar.dma_start(y_dram[:, b:b + 4, :], ob[:])
    nc.finalize()
    return nc


def _get_program():
    if "nc" not in _cache:
        consts = _host_consts()
        _cache["consts"] = consts
        _cache["nc"] = _build_nc(consts[0])
    return _cache["nc"], _cache["consts"]


def _in_maps(H):
    import ml_dtypes

    _, (u1, wmats) = _get_program()
    H3 = np.asarray(H, dtype=np.float32).reshape(G, G, D)
    Hp = np.zeros((G + 2, G, D), dtype=np.float32)
    Hp[1:G + 1] = H3
    in_maps = []
    for c in range(NC):
        shard = Hp[RPC * c: RPC * c + NR]                     # [34, 256, 512]
        xp = np.ascontiguousarray(
            shard.reshape(NR, 128, 2, D).transpose(1, 0, 2, 3)
        ).reshape(128, NR, G2).astype(ml_dtypes.bfloat16)
        in_maps.append({"x": xp, "wmat": wmats[c]})
    return in_maps


def _gather(results):
    outs = []
    for c in range(NC):
        y = np.asarray(results[c]["y"]).reshape(128, RPC, 2, D)
        y = y.transpose(1, 0, 2, 3).reshape(RPC * G, D).astype(np.float32)
        outs.append(y)
    return np.concatenate(outs, axis=0)


def kernel(H, xy=None):
    from concourse.bass_utils import run_bass_kernel_spmd

    nc, _ = _get_program()
    res = run_bass_kernel_spmd(nc, _in_maps(H), list(range(NC))).results
    return _gather(res)


# revision 34
# speedup vs baseline: 1.0039x; 1.0039x over previous
"""Localized embedding layer (Gaussian stencil) on 8 trn2 cores — bf16 pipeline.

Math: out[i,j,:] = sum_{|di|,|dj|<=2} w(di)w(dj) H[i+di,j+dj,:] / (r(i)*r(j))
with w(d) = exp(-c*d^2), c = TILE^2/(2 sigma^2). w(2) ~ 4.4e-5 is far below
the error budget, so the numerator collapses to a 3x3 stencil (the r(i)r(j)
normalizer keeps the exact 5-tap sums of the reference).

Sharding: 32 grid rows per core + 1-row halo each side (zero padded at the
global edges). All device I/O is bf16 (~0.3% rms error, budget 2e-2): this
problem is memory-bound and bf16 halves HBM traffic.

Layout: one grid row = one SBUF tile [128 partitions, 2 cells x 512 feat];
partition p holds cells 2p, 2p+1 (host pre-transposes shards to
partition-major [128, rows, 1024] so DMA lines are contiguous).

The j-conv runs on the PE as 128x128 matmuls whose matrices carry the exact
1/(w_full*r_col(j)) normalizer (so grid-edge columns need no fix pass); the
per-row normalizer w_full/r_row(i) (!=1 only for global rows 0,1,254,255) is
baked into dedicated per-core matrix sets for those rows, so the PSUM->SBUF
copy on ACT is scale-free and can process two rows per instruction.

The i-conv is balanced between DVE and PE per 4-row "quad":
  D-quads: DVE computes tt = (u1*x)[i-1] + (u1*x)[i+1] + x[i] as two packed
    2x-mode TTs over a prescaled z = u1*x chunk (avoids the 1x-mode STT);
    4 matmuls/row.
  P-quads: DVE computes only q = x[i-1]+x[i+1]; the PE folds u1 via extra
    matrix sets and consumes q and x[i] directly; 8 matmuls/row.
5 D / 3 P quads puts DVE and PE both at ~38us, under the ~45us DMA stream.
"""

import sys
import numpy as np

if "/opt/trn_rl_repo" not in sys.path:
    sys.path.insert(0, "/opt/trn_rl_repo")

G = 256          # grid side
D = 512          # feature dim
NC = 8           # cores
RPC = G // NC    # output rows per core = 32
NR = RPC + 2     # input rows per core incl 1-row halo = 34
G2 = 2 * D       # free size of one grid row tile (2 cells x 512)
TILE = 448.0
SIGMA = 200.0
P5 = 2           # reference stencil half-width (for the normalizer r)
# matrix sets: std, u1*std, row0, row1, row30, row31, u1*row0, u1*row1
NSET = 8

# input DMA row ranges in the 34-row shard (first one tiny so the very first
# compute — a 2-row P piece needing only rows 0..3 — starts ~2us earlier);
# subtile dependency tracking lets every consumer wait on exactly the loads
# covering its rows, so ranges don't overlap.
LOADS = [(0, 4), (4, 12), (12, 20), (20, 28), (28, 34)]
# P-quads first: their raw TT needs no z-prescale, so the PE pipeline kicks
# off as early as possible. (Making the LAST quad P-scheme was tried and is
# ~8us WORSE: the +32 matmuls grow the PE stream by more than the drain
# saves.)
SCHEME = ["P", "P", "D", "D", "D", "D", "D", "D"]
# z = u1*x ranges (cover side-tap rows of D-quads only), split at LOAD
# boundaries so no TS op waits on a later DMA than its consumers need; the
# z tile holds rows 8..33 at index row-8.
ZRANGES = [(8, 12), (12, 20), (20, 28), (28, 34)]
ZROWS = 26


def _zi(r):
    return r - 8

_cache = {}


def _weights5():
    c = TILE * TILE / (2.0 * SIGMA * SIGMA)
    return np.exp(-c * np.arange(-P5, P5 + 1) ** 2)   # [w2,w1,1,w1,w2] f64


def _r_vec():
    """r(i) = sum of valid 5-tap weights at row/col i (reference normalizer)."""
    w = _weights5()
    r = np.zeros(G)
    for d in range(-P5, P5 + 1):
        lo, hi = max(0, -d), min(G, G - d)
        r[lo:hi] += w[d + P5]
    return r


def _host_consts():
    import ml_dtypes

    w = _weights5()
    u1 = float(w[1])
    r = _r_vec()
    wf = float(w.sum())
    # Base j-conv matrices, m[q, k, p]: ps_c[p] = sum_q m[q,k(c),p] * rhs[q]
    base = np.zeros((128, 4, 128))
    for p in range(128):
        s0 = 1.0 / (wf * r[2 * p])
        s1 = 1.0 / (wf * r[2 * p + 1])
        base[p, 0, p] = s0                      # D0: c0 center
        base[p, 1, p] = u1 * s0                 # B: c0 <- c1 (cell 2p+1)
        if p > 0:
            base[p - 1, 1, p] = u1 * s0         # B: c0 <- c1 (cell 2p-1)
        base[p, 2, p] = u1 * s1                 # C: c1 <- c0 (cell 2p)
        if p < 127:
            base[p + 1, 2, p] = u1 * s1         # C: c1 <- c0 (cell 2p+2)
        base[p, 3, p] = s1                      # D1: c1 center
    # Per-core matrix sets: [std, u1*std, s_row0*std, s_row1*std,
    #                        s_row30*std, s_row31*std, u1*s_row0*std,
    #                        u1*s_row1*std]
    wmats = []
    for c in range(NC):
        sets = [base, u1 * base]
        for i in (0, 1, RPC - 2, RPC - 1):
            s = wf / r[RPC * c + i]             # ==1 away from global edges
            sets.append(s * base)
        for i in (0, 1):
            s = wf / r[RPC * c + i]
            sets.append(u1 * s * base)
        wm = np.concatenate(sets, axis=1)        # [128, 32, 128]
        wmats.append(wm.astype(ml_dtypes.bfloat16))
    return u1, wmats


def _kset(i):
    """Matrix-set index for output row i (0=std; edge rows pre-scaled)."""
    return {0: 2, 1: 3, RPC - 2: 4, RPC - 1: 5}.get(i, 0)


def _build_nc(u1):
    import concourse.bass as bass  # noqa: F401
    import concourse.mybir as mybir
    import concourse.tile as tile
    from concourse import bacc

    f32 = mybir.dt.float32
    bf16 = mybir.dt.bfloat16
    add = mybir.AluOpType.add

    nc = bacc.Bacc(None, target_bir_lowering=False, debug=False)
    x_dram = nc.declare_dram_parameter("x", [128, NR, G2], bf16, isOutput=False)
    wm_dram = nc.declare_dram_parameter(
        "wmat", [128, 4 * NSET, 128], bf16, isOutput=False
    )
    y_dram = nc.declare_dram_parameter("y", [128, RPC, G2], bf16, isOutput=True)

    with tile.TileContext(nc) as tc:
        with (
            tc.tile_pool(name="const", bufs=1) as cpool,
            tc.tile_pool(name="x", bufs=1) as xpool,
            tc.tile_pool(name="z", bufs=1) as zpool,
            tc.tile_pool(name="qp", bufs=4) as qpool,
            tc.tile_pool(name="ttp", bufs=3) as ttpool,
            tc.tile_pool(name="out", bufs=2) as opool,
            tc.tile_pool(name="psum", bufs=2, space="PSUM") as ppool,
        ):
            # consts ride the scalar ring so they don't delay the first load
            wt = cpool.tile([128, 4 * NSET, 128], bf16)
            nc.scalar.dma_start(wt[:], wm_dram[:])

            xt = xpool.tile([128, NR, G2], bf16, name="xt")
            for s, e in LOADS:
                nc.sync.dma_start(xt[:, s:e, :], x_dram[:, s:e, :])
            zt = zpool.tile([128, ZROWS, G2], bf16, name="zt")
            for s, e in ZRANGES:
                nc.vector.tensor_scalar_mul(
                    zt[:, _zi(s):_zi(s) + (e - s), :], xt[:, s:e, :], u1
                )

            def d_pair(ob, ttq, tb, q, mm, dve_copy=False):
                """ttq rows [tb, tb+1] -> output rows 4q+mm, 4q+mm+1."""
                ps = ppool.tile([128, 2 * G2], f32, tag="ps", name="ps")
                for rr in (0, 1):
                    k = 4 * _kset(4 * q + mm + rr)
                    o = rr * G2
                    rhs0 = ttq[:, tb + rr, 0:D]
                    rhs1 = ttq[:, tb + rr, D:G2]
                    nc.tensor.matmul(
                        ps[:, o:o + D], wt[:, k + 0, :], rhs0,
                        start=True, stop=False,
                    )
                    nc.tensor.matmul(
                        ps[:, o:o + D], wt[:, k + 1, :], rhs1,
                        start=False, stop=True,
                    )
                    nc.tensor.matmul(
                        ps[:, o + D:o + G2], wt[:, k + 2, :], rhs0,
                        start=True, stop=False,
                    )
                    nc.tensor.matmul(
                        ps[:, o + D:o + G2], wt[:, k + 3, :], rhs1,
                        start=False, stop=True,
                    )
                if dve_copy:
                    # DVE is idle during the drain; halves the ACT tail
                    nc.vector.tensor_copy(ob[:, mm:mm + 2, :], ps[:])
                else:
                    nc.scalar.copy(ob[:, mm:mm + 2, :], ps[:])

            def d_ttq(b):
                """DVE i-conv for a D-quad starting at input row b."""
                zb = _zi(b)
                qt = qpool.tile([128, 4, G2], bf16, tag="qt", name="qt")
                nc.vector.tensor_tensor(
                    qt[:], zt[:, zb:zb + 4, :], zt[:, zb + 2:zb + 6, :], add
                )
                ttq = ttpool.tile([128, 4, G2], bf16, tag="tt", name="ttq")
                nc.vector.tensor_tensor(
                    ttq[:], qt[:], xt[:, b + 1:b + 5, :], add
                )
                return ttq

            def p_pair(ob, qt, tb, i0, om):
                """P-scheme: qt rows [tb, tb+1] -> output rows i0, i0+1
                (written to ob rows om, om+1)."""
                ps = ppool.tile([128, 2 * G2], f32, tag="ps", name="ps")
                for rr in (0, 1):
                    i = i0 + rr
                    kc = 4 * _kset(i)              # center sets
                    ku = {0: 24, 1: 28}.get(i, 4)  # u1-scaled side sets
                    o = rr * G2
                    qa, qb = qt[:, tb + rr, 0:D], qt[:, tb + rr, D:G2]
                    x1 = xt[:, i + 1, :]
                    for (lo, hi), s in (((0, D), 0), ((D, G2), 2)):
                        nc.tensor.matmul(
                            ps[:, o + lo:o + hi], wt[:, ku + s, :], qa,
                            start=True, stop=False,
                        )
                        nc.tensor.matmul(
                            ps[:, o + lo:o + hi], wt[:, ku + 1 + s, :],
                            qb, start=False, stop=False,
                        )
                        nc.tensor.matmul(
                            ps[:, o + lo:o + hi], wt[:, kc + s, :],
                            x1[:, 0:D], start=False, stop=False,
                        )
                        nc.tensor.matmul(
                            ps[:, o + lo:o + hi], wt[:, kc + 1 + s, :],
                            x1[:, D:G2], start=False, stop=True,
                        )
                nc.scalar.copy(ob[:, om:om + 2, :], ps[:])

            for q in range(8):
                b = 4 * q                      # first input row of the quad
                if q == 0:
                    # first quad in 2-row pieces: the first piece needs only
                    # input rows 0..3 (the tiny first load), so PE starts
                    # ~2us earlier
                    ob = opool.tile([128, 4, G2], bf16, tag="ob", name="ob")
                    for mm in (0, 2):
                        qt = qpool.tile([128, 2, G2], bf16, tag="qt", name="qt")
                        nc.vector.tensor_tensor(
                            qt[:], xt[:, mm:mm + 2, :], xt[:, mm + 2:mm + 4, :],
                            add,
                        )
                        p_pair(ob, qt, 0, mm, mm)
                    nc.scalar.dma_start(y_dram[:, 0:4, :], ob[:])
                    continue
                if q == 7:
                    # final quad: per-pair stores (last on the idle sync ring)
                    # shorten the drain tail
                    ttq = d_ttq(b)
                    ob7 = opool.tile([128, 4, G2], bf16, tag="ob", name="ob")
                    d_pair(ob7, ttq, 0, 7, 0)
                    nc.scalar.dma_start(y_dram[:, 28:30, :], ob7[:, 0:2, :])
                    d_pair(ob7, ttq, 2, 7, 2)
                    nc.sync.dma_start(y_dram[:, 30:32, :], ob7[:, 2:4, :])
                    break
                ob = opool.tile([128, 4, G2], bf16, tag="ob", name="ob")
                if SCHEME[q] == "D":
                    ttq = d_ttq(b)
                    for mm in (0, 2):
                        d_pair(ob, ttq, mm, q, mm)
                    nc.scalar.dma_start(y_dram[:, b:b + 4, :], ob[:])
                else:
                    # P-quad: q_raw = x[i-1] + x[i+1]; PE folds u1 and center
                    qt = qpool.tile([128, 4, G2], bf16, tag="qt", name="qt")
                    nc.vector.tensor_tensor(
                        qt[:], xt[:, b:b + 4, :], xt[:, b + 2:b + 6, :], add
                    )
                    for mm in (0, 2):
                        p_pair(ob, qt, mm, 4 * q + mm, mm)
                    nc.scalar.dma_start(y_dram[:, b:b + 4, :], ob[:])
    nc.finalize()
    return nc


def _get_program():
    if "nc" not in _cache:
        consts = _host_consts()
        _cache["consts"] = consts
        _cache["nc"] = _build_nc(consts[0])
    return _cache["nc"], _cache["consts"]


def _in_maps(H):
    import ml_dtypes

    _, (u1, wmats) = _get_program()
    H3 = np.asarray(H, dtype=np.float32).reshape(G, G, D)
    Hp = np.zeros((G + 2, G, D), dtype=np.float32)
    Hp[1:G + 1] = H3
    in_maps = []
    for c in range(NC):
        shard = Hp[RPC * c: RPC * c + NR]                     # [34, 256, 512]
        xp = np.ascontiguousarray(
            shard.reshape(NR, 128, 2, D).transpose(1, 0, 2, 3)
        ).reshape(128, NR, G2).astype(ml_dtypes.bfloat16)
        in_maps.append({"x": xp, "wmat": wmats[c]})
    return in_maps


def _gather(results):
    outs = []
    for c in range(NC):
        y = np.asarray(results[c]["y"]).reshape(128, RPC, 2, D)
        y = y.transpose(1, 0, 2, 3).reshape(RPC * G, D).astype(np.float32)
        outs.append(y)
    return np.concatenate(outs, axis=0)


def kernel(H, xy=None):
    from concourse.bass_utils import run_bass_kernel_spmd

    nc, _ = _get_program()
    res = run_bass_kernel_spmd(nc, _in_maps(H), list(range(NC))).results
    return _gather(res)


# revision 35
# speedup vs baseline: 1.1850x; 1.1804x over previous
"""Localized embedding layer (Gaussian stencil) on 8 trn2 cores — bf16 pipeline.

Math: out[i,j,:] = sum_{|di|,|dj|<=2} w(di)w(dj) H[i+di,j+dj,:] / (r(i)*r(j))
with w(d) = exp(-c*d^2), c = TILE^2/(2 sigma^2). w(2) ~ 4.4e-5 is far below
the error budget, so the numerator collapses to a 3x3 stencil (the r(i)r(j)
normalizer keeps the exact 5-tap sums of the reference).

Sharding: 32 grid rows per core + 1-row halo each side (zero padded at the
global edges). All device I/O is bf16 (~0.3% rms error, budget 2e-2): this
problem is memory-bound and bf16 halves HBM traffic.

Layout: one grid row = one SBUF tile [128 partitions, 2 cells x 512 feat];
partition p holds cells 2p, 2p+1 (host pre-transposes shards to
partition-major [128, rows, 1024] so DMA lines are contiguous).

The j-conv runs on the PE as 128x128 matmuls whose matrices carry the exact
1/(w_full*r_col(j)) normalizer (so grid-edge columns need no fix pass); the
per-row normalizer w_full/r_row(i) (!=1 only for global rows 0,1,254,255) is
baked into dedicated per-core matrix sets for those rows, so the PSUM->SBUF
copy on ACT is scale-free and can process two rows per instruction.

The i-conv is balanced between DVE and PE per 4-row "quad":
  D-quads: DVE computes tt = (u1*x)[i-1] + (u1*x)[i+1] + x[i] as two packed
    2x-mode TTs over a prescaled z = u1*x chunk (avoids the 1x-mode STT);
    4 matmuls/row.
  P-quads: DVE computes only q = x[i-1]+x[i+1]; the PE folds u1 via extra
    matrix sets and consumes q and x[i] directly; 8 matmuls/row.
5 D / 3 P quads puts DVE and PE both at ~38us, under the ~45us DMA stream.
"""

import sys
import numpy as np

if "/opt/trn_rl_repo" not in sys.path:
    sys.path.insert(0, "/opt/trn_rl_repo")

G = 256          # grid side
D = 512          # feature dim
NC = 8           # cores
RPC = G // NC    # output rows per core = 32
NR = RPC + 2     # input rows per core incl 1-row halo = 34
G2 = 2 * D       # free size of one grid row tile (2 cells x 512)
TILE = 448.0
SIGMA = 200.0
P5 = 2           # reference stencil half-width (for the normalizer r)
# matrix sets: std, u1*std, row0, row1, row30, row31, u1*row0, u1*row1
NSET = 8

# input DMA row ranges in the 34-row shard (first one tiny so the very first
# compute — a 2-row P piece needing only rows 0..3 — starts ~2us earlier);
# subtile dependency tracking lets every consumer wait on exactly the loads
# covering its rows, so ranges don't overlap.
LOADS = [(0, 4), (4, 8), (8, 12), (12, 20), (20, 28), (28, 34)]
# P-quads first: their raw TT needs no z-prescale, so the PE pipeline kicks
# off as early as possible. (Making the LAST quad P-scheme was tried and is
# ~8us WORSE: the +32 matmuls grow the PE stream by more than the drain
# saves.)
SCHEME = ["P", "P", "D", "D", "D", "D", "D", "D"]
# z = u1*x ranges (cover side-tap rows of D-quads only), split at LOAD
# boundaries so no TS op waits on a later DMA than its consumers need; the
# z tile holds rows 8..33 at index row-8.
ZRANGES = [(8, 12), (12, 20), (20, 28), (28, 34)]
ZROWS = 26


def _zi(r):
    return r - 8

_cache = {}


def _weights5():
    c = TILE * TILE / (2.0 * SIGMA * SIGMA)
    return np.exp(-c * np.arange(-P5, P5 + 1) ** 2)   # [w2,w1,1,w1,w2] f64


def _r_vec():
    """r(i) = sum of valid 5-tap weights at row/col i (reference normalizer)."""
    w = _weights5()
    r = np.zeros(G)
    for d in range(-P5, P5 + 1):
        lo, hi = max(0, -d), min(G, G - d)
        r[lo:hi] += w[d + P5]
    return r


def _host_consts():
    import ml_dtypes

    w = _weights5()
    u1 = float(w[1])
    r = _r_vec()
    wf = float(w.sum())
    # Base j-conv matrices, m[q, k, p]: ps_c[p] = sum_q m[q,k(c),p] * rhs[q]
    base = np.zeros((128, 4, 128))
    for p in range(128):
        s0 = 1.0 / (wf * r[2 * p])
        s1 = 1.0 / (wf * r[2 * p + 1])
        base[p, 0, p] = s0                      # D0: c0 center
        base[p, 1, p] = u1 * s0                 # B: c0 <- c1 (cell 2p+1)
        if p > 0:
            base[p - 1, 1, p] = u1 * s0         # B: c0 <- c1 (cell 2p-1)
        base[p, 2, p] = u1 * s1                 # C: c1 <- c0 (cell 2p)
        if p < 127:
            base[p + 1, 2, p] = u1 * s1         # C: c1 <- c0 (cell 2p+2)
        base[p, 3, p] = s1                      # D1: c1 center
    # Per-core matrix sets: [std, u1*std, s_row0*std, s_row1*std,
    #                        s_row30*std, s_row31*std, u1*s_row0*std,
    #                        u1*s_row1*std]
    wmats = []
    for c in range(NC):
        sets = [base, u1 * base]
        for i in (0, 1, RPC - 2, RPC - 1):
            s = wf / r[RPC * c + i]             # ==1 away from global edges
            sets.append(s * base)
        for i in (0, 1):
            s = wf / r[RPC * c + i]
            sets.append(u1 * s * base)
        wm = np.concatenate(sets, axis=1)        # [128, 32, 128]
        wmats.append(wm.astype(ml_dtypes.bfloat16))
    return u1, wmats


def _kset(i):
    """Matrix-set index for output row i (0=std; edge rows pre-scaled)."""
    return {0: 2, 1: 3, RPC - 2: 4, RPC - 1: 5}.get(i, 0)


def _build_nc(u1):
    import concourse.bass as bass  # noqa: F401
    import concourse.mybir as mybir
    import concourse.tile as tile
    from concourse import bacc

    f32 = mybir.dt.float32
    bf16 = mybir.dt.bfloat16
    add = mybir.AluOpType.add

    nc = bacc.Bacc(None, target_bir_lowering=False, debug=False)
    x_dram = nc.declare_dram_parameter("x", [128, NR, G2], bf16, isOutput=False)
    wm_dram = nc.declare_dram_parameter(
        "wmat", [128, 4 * NSET, 128], bf16, isOutput=False
    )
    y_dram = nc.declare_dram_parameter("y", [128, RPC, G2], bf16, isOutput=True)

    with tile.TileContext(nc) as tc:
        with (
            tc.tile_pool(name="const", bufs=1) as cpool,
            tc.tile_pool(name="x", bufs=1) as xpool,
            tc.tile_pool(name="z", bufs=1) as zpool,
            tc.tile_pool(name="qp", bufs=4) as qpool,
            tc.tile_pool(name="ttp", bufs=3) as ttpool,
            tc.tile_pool(name="out", bufs=2) as opool,
            tc.tile_pool(name="psum", bufs=2, space="PSUM") as ppool,
        ):
            # consts ride the scalar ring so they don't delay the first load
            wt = cpool.tile([128, 4 * NSET, 128], bf16)
            nc.scalar.dma_start(wt[:], wm_dram[:])

            xt = xpool.tile([128, NR, G2], bf16, name="xt")
            for s, e in LOADS:
                nc.sync.dma_start(xt[:, s:e, :], x_dram[:, s:e, :])
            zt = zpool.tile([128, ZROWS, G2], bf16, name="zt")
            for s, e in ZRANGES:
                nc.vector.tensor_scalar_mul(
                    zt[:, _zi(s):_zi(s) + (e - s), :], xt[:, s:e, :], u1
                )

            def d_pair(ob, ttq, tb, q, mm, dve_copy=False):
                """ttq rows [tb, tb+1] -> output rows 4q+mm, 4q+mm+1."""
                ps = ppool.tile([128, 2 * G2], f32, tag="ps", name="ps")
                for rr in (0, 1):
                    k = 4 * _kset(4 * q + mm + rr)
                    o = rr * G2
                    rhs0 = ttq[:, tb + rr, 0:D]
                    rhs1 = ttq[:, tb + rr, D:G2]
                    nc.tensor.matmul(
                        ps[:, o:o + D], wt[:, k + 0, :], rhs0,
                        start=True, stop=False,
                    )
                    nc.tensor.matmul(
                        ps[:, o:o + D], wt[:, k + 1, :], rhs1,
                        start=False, stop=True,
                    )
                    nc.tensor.matmul(
                        ps[:, o + D:o + G2], wt[:, k + 2, :], rhs0,
                        start=True, stop=False,
                    )
                    nc.tensor.matmul(
                        ps[:, o + D:o + G2], wt[:, k + 3, :], rhs1,
                        start=False, stop=True,
                    )
                if dve_copy:
                    # DVE is idle during the drain; halves the ACT tail
                    nc.vector.tensor_copy(ob[:, mm:mm + 2, :], ps[:])
                else:
                    nc.scalar.copy(ob[:, mm:mm + 2, :], ps[:])

            def d_ttq(b):
                """DVE i-conv for a D-quad starting at input row b."""
                zb = _zi(b)
                qt = qpool.tile([128, 4, G2], bf16, tag="qt", name="qt")
                nc.vector.tensor_tensor(
                    qt[:], zt[:, zb:zb + 4, :], zt[:, zb + 2:zb + 6, :], add
                )
                ttq = ttpool.tile([128, 4, G2], bf16, tag="tt", name="ttq")
                nc.vector.tensor_tensor(
                    ttq[:], qt[:], xt[:, b + 1:b + 5, :], add
                )
                return ttq

            def p_pair(ob, qt, tb, i0, om):
                """P-scheme: qt rows [tb, tb+1] -> output rows i0, i0+1
                (written to ob rows om, om+1)."""
                ps = ppool.tile([128, 2 * G2], f32, tag="ps", name="ps")
                for rr in (0, 1):
                    i = i0 + rr
                    kc = 4 * _kset(i)              # center sets
                    ku = {0: 24, 1: 28}.get(i, 4)  # u1-scaled side sets
                    o = rr * G2
                    qa, qb = qt[:, tb + rr, 0:D], qt[:, tb + rr, D:G2]
                    x1 = xt[:, i + 1, :]
                    for (lo, hi), s in (((0, D), 0), ((D, G2), 2)):
                        nc.tensor.matmul(
                            ps[:, o + lo:o + hi], wt[:, ku + s, :], qa,
                            start=True, stop=False,
                        )
                        nc.tensor.matmul(
                            ps[:, o + lo:o + hi], wt[:, ku + 1 + s, :],
                            qb, start=False, stop=False,
                        )
                        nc.tensor.matmul(
                            ps[:, o + lo:o + hi], wt[:, kc + s, :],
                            x1[:, 0:D], start=False, stop=False,
                        )
                        nc.tensor.matmul(
                            ps[:, o + lo:o + hi], wt[:, kc + 1 + s, :],
                            x1[:, D:G2], start=False, stop=True,
                        )
                nc.scalar.copy(ob[:, om:om + 2, :], ps[:])

            for q in range(8):
                b = 4 * q                      # first input row of the quad
                if q == 0:
                    # first quad in 2-row pieces: the first piece needs only
                    # input rows 0..3 (the tiny first load), so PE starts
                    # ~2us earlier
                    ob = opool.tile([128, 4, G2], bf16, tag="ob", name="ob")
                    for mm in (0, 2):
                        qt = qpool.tile([128, 2, G2], bf16, tag="qt", name="qt")
                        nc.vector.tensor_tensor(
                            qt[:], xt[:, mm:mm + 2, :], xt[:, mm + 2:mm + 4, :],
                            add,
                        )
                        p_pair(ob, qt, 0, mm, mm)
                    nc.scalar.dma_start(y_dram[:, 0:4, :], ob[:])
                    continue
                if q == 7:
                    # final quad: per-pair stores (last on the idle sync ring)
                    # shorten the drain tail
                    ttq = d_ttq(b)
                    ob7 = opool.tile([128, 4, G2], bf16, tag="ob", name="ob")
                    d_pair(ob7, ttq, 0, 7, 0)
                    nc.scalar.dma_start(y_dram[:, 28:30, :], ob7[:, 0:2, :])
                    d_pair(ob7, ttq, 2, 7, 2)
                    nc.sync.dma_start(y_dram[:, 30:32, :], ob7[:, 2:4, :])
                    break
                ob = opool.tile([128, 4, G2], bf16, tag="ob", name="ob")
                if SCHEME[q] == "D":
                    ttq = d_ttq(b)
                    for mm in (0, 2):
                        d_pair(ob, ttq, mm, q, mm)
                    nc.scalar.dma_start(y_dram[:, b:b + 4, :], ob[:])
                else:
                    # P-quad: q_raw = x[i-1] + x[i+1]; PE folds u1 and center
                    qt = qpool.tile([128, 4, G2], bf16, tag="qt", name="qt")
                    nc.vector.tensor_tensor(
                        qt[:], xt[:, b:b + 4, :], xt[:, b + 2:b + 6, :], add
                    )
                    for mm in (0, 2):
                        p_pair(ob, qt, mm, 4 * q + mm, mm)
                    nc.scalar.dma_start(y_dram[:, b:b + 4, :], ob[:])
    nc.finalize()
    return nc


def _get_program():
    if "nc" not in _cache:
        consts = _host_consts()
        _cache["consts"] = consts
        _cache["nc"] = _build_nc(consts[0])
    return _cache["nc"], _cache["consts"]


def _in_maps(H):
    import ml_dtypes

    _, (u1, wmats) = _get_program()
    H3 = np.asarray(H, dtype=np.float32).reshape(G, G, D)
    Hp = np.zeros((G + 2, G, D), dtype=np.float32)
    Hp[1:G + 1] = H3
    in_maps = []
    for c in range(NC):
        shard = Hp[RPC * c: RPC * c + NR]                     # [34, 256, 512]
        xp = np.ascontiguousarray(
            shard.reshape(NR, 128, 2, D).transpose(1, 0, 2, 3)
        ).reshape(128, NR, G2).astype(ml_dtypes.bfloat16)
        in_maps.append({"x": xp, "wmat": wmats[c]})
    return in_maps


def _gather(results):
    outs = []
    for c in range(NC):
        y = np.asarray(results[c]["y"]).reshape(128, RPC, 2, D)
        y = y.transpose(1, 0, 2, 3).reshape(RPC * G, D).astype(np.float32)
        outs.append(y)
    return np.concatenate(outs, axis=0)


def kernel(H, xy=None):
    from concourse.bass_utils import run_bass_kernel_spmd

    nc, _ = _get_program()
    res = run_bass_kernel_spmd(nc, _in_maps(H), list(range(NC))).results
    return _gather(res)
